# revision 47
# baseline (speedup 1.0000x reference)
"""GAT (2-layer graph attention network) Trainium2 kernel, bf16 m-major edition.

Contract: kernel(**inputs) takes the FULL inputs from setup_inputs() and
returns the full (32, 256, 512) float32 output. Internally shards the batch
across 8 NeuronCores (4 graphs per core), runs a Bass/Tile kernel per core,
and concatenates the results.

Design notes:
  * All matmul operands are bf16 (FWL weight loads, 1 cyc/row); PSUM
    accumulation is f32. Post-exp attention weights are bf16; logits are
    computed with f32 per-partition scalars folded in via fused DVE ops.
  * Softmax is M-MAJOR: the exp tensor is produced already transposed
    (p[m-part, n-free]) so NO PE transposes / diag matmuls are needed.
    Row sums z[n] come from two ones-column matmuls (partition reduction),
    1/z is broadcast to all partitions with a stride-0 SBUF->SBUF DMA,
    and normalization is one DVE multiply.
  * logit chain: x = (mnegT + e2col) + e1row via ONE fused
    scalar_tensor_tensor per m-chunk; leaky = (x*0.2) max x via one more.
    exp is a single [128, 512] scalar pass per head (no bias, no accum).
  * e1/e2 for layer 1 are host-gathered from folded tables (exact),
    shipped as per-partition f32 columns (e2) and bf16 broadcast rows (e1).
  * elu(x) = max(min(exp(x)-1, 0), x): one scalar Exp + one fused DVE
    (sub,min) + one DVE max against the raw PSUM value.
  * A warmup burst of identity transposes at kernel start brings the PE
    out of its HAM 1.2 GHz throttle state before real matmuls begin.
  Softmax skips max-subtraction: logits are O(1) for real entries; masked
  entries get -1000 so exp underflows to 0, matching where(mask, e, -9e15).
"""

import numpy as np
from contextlib import ExitStack
import ml_dtypes

import concourse.bass as bass
import concourse.tile as tile
from concourse import mybir, bacc
from concourse.bass_utils import run_bass_kernel_spmd

f32 = mybir.dt.float32
bf16 = mybir.dt.bfloat16
AF = mybir.ActivationFunctionType
AL = mybir.AluOpType
BF = ml_dtypes.bfloat16

# Problem dims (hardcoded per contract)
B, N, VOCAB, F, O, H, OUT = 32, 256, 200, 300, 256, 8, 512
NCORES = 8
GPC = B // NCORES          # graphs per core
NC = N // 128              # node chunks (2)
VC = 2                     # vocab chunks (padded 200->256)
HP = H // 2                # head pairs (aug matmuls produce 2 heads/bank)
KC2 = (H * O) // 128       # layer-2 contraction chunks (16)
ALPHA = 0.2
MASK_NEG = -1000.0


def _build_nc():
    nc = bacc.Bacc("TRN2", target_bir_lowering=False, debug=False,
                   num_devices=NCORES)

    oh_d = nc.dram_tensor("oh", [GPC, 128, VC, N], bf16, kind="ExternalInput").ap()
    mnegt_d = nc.dram_tensor("mnegt", [GPC, 128, NC, N], bf16, kind="ExternalInput").ap()
    e2c_d = nc.dram_tensor("e2c", [GPC, 128, NC, H], f32, kind="ExternalInput").ap()
    e1bc_d = nc.dram_tensor("e1bc", [GPC, 128, H, N], bf16, kind="ExternalInput").ap()
    npm_d = nc.dram_tensor("npm", [GPC, 128, NC], f32, kind="ExternalInput").ap()
    zscr_d = nc.dram_tensor("zscr", [GPC, H + 2, N], bf16, kind="Internal").ap()
    ew_d = nc.dram_tensor("embw2", [128, VC, HP, 2 * O], bf16, kind="ExternalInput").ap()
    wo_d = nc.dram_tensor("woaug", [128, KC2, 2, 258], bf16, kind="ExternalInput").ap()
    idn_d = nc.dram_tensor("identity", [128, 128], f32, kind="ExternalInput").ap()
    out_d = nc.dram_tensor("out", [GPC, 128, NC, OUT], bf16, kind="ExternalOutput").ap()

    with tile.TileContext(nc) as tc, ExitStack() as ctx:
        const = ctx.enter_context(tc.tile_pool(name="const", bufs=1))
        gpool = ctx.enter_context(tc.tile_pool(name="gpool", bufs=3))
        hpool = ctx.enter_context(tc.tile_pool(name="hpool", bufs=4))
        hbig = ctx.enter_context(tc.tile_pool(name="hbig", bufs=4))
        ps_a = ctx.enter_context(tc.tile_pool(name="ps_a", bufs=3, space="PSUM"))
        ps_b = ctx.enter_context(tc.tile_pool(name="ps_b", bufs=3, space="PSUM"))
        ps_z = ctx.enter_context(tc.tile_pool(name="ps_z", bufs=2, space="PSUM"))

        # ---- resident constants ----
        ident = const.tile([128, 128], f32)
        nc.sync.dma_start(ident[:], idn_d)
        ew_sb = const.tile([128, VC, HP, 2 * O], bf16)
        for vc in range(VC):
            nc.sync.dma_start(ew_sb[:, vc].rearrange("p a b -> p (a b)"),
                              ew_d[:, vc].rearrange("p a b -> p (a b)"))
        wo_sb = const.tile([128, KC2, 2, 258], bf16)
        for k in range(KC2):
            nc.sync.dma_start(wo_sb[:, k].rearrange("p a b -> p (a b)"),
                              wo_d[:, k].rearrange("p a b -> p (a b)"))
        ones_col = const.tile([128, 1], bf16)
        nc.vector.memset(ones_col[:], 1.0)

        # warm up the PE (HAM un-throttle needs ~3.5us of sustained work)
        for w in range(20):
            wps = ps_b.tile([128, 128], f32, tag="big")
            nc.tensor.transpose(wps[:], ident[:], ident[:])

        G = {}

        def bc_nc(ap2, n=NC):
            # view a [128, X] AP as [128, n, X] via a stride-0 middle dim
            return bass.AP(tensor=ap2.tensor, offset=ap2.offset,
                           ap=[ap2.ap[0], [0, n], ap2.ap[-1]])

        def emit_setup(g):
            s = G[g] = {}
            oh_sb = gpool.tile([128, VC, N], bf16, tag="oh")
            nc.sync.dma_start(oh_sb[:], oh_d[g])
            mnegt = gpool.tile([128, NC, N], bf16, tag="mnegt")
            nc.sync.dma_start(mnegt[:], mnegt_d[g])
            e2c = gpool.tile([128, NC, H], f32, tag="e2c")
            nc.sync.dma_start(e2c[:], e2c_d[g])
            e1bc = gpool.tile([128, H, N], bf16, tag="e1bc")
            nc.sync.dma_start(e1bc[:], e1bc_d[g])
            npm_sb = gpool.tile([128, NC], f32, tag="npm")
            nc.sync.dma_start(npm_sb[:], npm_d[g])
            hT = hbig.tile([128, KC2, N], bf16)
            s.update(oh_sb=oh_sb, mnegt=mnegt, e2c=e2c, e1bc=e1bc,
                     npm=npm_sb, hT=hT, wh={})

        def emit_aug(g, hp):
            # one PSUM bank holds Wh for a HEAD PAIR (2*O wide) per chunk
            s = G[g]
            whp = hpool.tile([128, NC, 2, O], bf16, tag="wh")
            s["wh"][hp] = whp
            for c in range(NC):
                aug = ps_a.tile([128, 2 * O], f32, tag="aug")
                for vc in range(VC):
                    nc.tensor.matmul(
                        aug[:], lhsT=s["oh_sb"][:, vc, c * 128:(c + 1) * 128],
                        rhs=ew_sb[:, vc, hp, :],
                        start=(vc == 0), stop=(vc == VC - 1))
                nc.scalar.copy(
                    whp[:, c].rearrange("p a b -> p (a b)"), aug[:])

        def emit_head_pair(g, hp):
            s = G[g]
            whp = s["wh"].pop(hp)
            mnegt, e2c, e1bc, hT = s["mnegt"], s["e2c"], s["e1bc"], s["hT"]
            # x[m, n] = mnegT + e2[m] + e1[n], head pair in one tile
            x = hpool.tile([128, 2, NC, N], bf16, tag="x")
            for hi in range(2):
                h = 2 * hp + hi
                for mc in range(NC):
                    nc.vector.scalar_tensor_tensor(
                        x[:, hi, mc, :], mnegt[:, mc, :],
                        e2c[:, mc, h:h + 1], e1bc[:, h, :],
                        op0=AL.add, op1=AL.add)
            zt = hpool.tile([128, 2, NC, N], bf16, tag="zt")
            nc.vector.scalar_tensor_tensor(
                zt[:].rearrange("p a b c -> p (a b c)"),
                x[:].rearrange("p a b c -> p (a b c)"), ALPHA,
                x[:].rearrange("p a b c -> p (a b c)"),
                op0=AL.mult, op1=AL.max)
            p_u = {}
            for hi in range(2):
                pu = hpool.tile([128, NC, N], bf16, tag="pu")
                p_u[hi] = pu
                nc.scalar.activation(pu[:].rearrange("p a b -> p (a b)"),
                                     zt[:, hi].rearrange("p a b -> p (a b)"),
                                     AF.Exp)
            at = hpool.tile([128, 2, NC, O], bf16, tag="at")
            opss = {}
            for hi in range(2):
                h = 2 * hp + hi
                # z[n] as COLUMNS (n on partitions): reciprocal runs 128-wide
                zc = ps_z.tile([128, NC], f32, tag="z")
                for c in range(NC):
                    for mc in range(NC):
                        nc.tensor.matmul(
                            zc[:, c:c + 1],
                            lhsT=p_u[hi][:, mc, c * 128:(c + 1) * 128],
                            rhs=ones_col[:],
                            start=(mc == 0), stop=(mc == NC - 1))
                zic = hpool.tile([128, NC], bf16, tag="zic")
                with nc.allow_low_precision("bf16 softmax 1/z"):
                    nc.vector.reciprocal(zic[:], zc[:])
                # broadcast 1/z to every partition via a DRAM round-trip
                # (stride-0 read replicates the 256-value row 128x)
                zs = zscr_d[g, h]
                nc.sync.dma_start(
                    bass.AP(tensor=zs.tensor, offset=zs.offset,
                            ap=[[1, 128], [128, NC]]), zic[:])
                zbcs = hpool.tile([128, N], bf16, tag="zbc")
                nc.sync.dma_start(
                    zbcs[:], bass.AP(tensor=zs.tensor, offset=zs.offset,
                                     ap=[[0, 128], [1, N]]))
                pn = hpool.tile([128, NC, N], bf16, tag="pn")
                nc.vector.tensor_tensor(pn[:], p_u[hi][:], bc_nc(zbcs[:]),
                                        op=AL.mult)
                # out1T[o, n] = Wh.T @ pn  (o-major: ready for layer 2)
                ops = ps_b.tile([128, 2 * O], f32, tag="big")
                opss[hi] = ops
                for oc in range(NC):
                    for mc in range(NC):
                        nc.tensor.matmul(
                            ops[:, oc * O:(oc + 1) * O],
                            lhsT=whp[:, mc, hi, oc * 128:(oc + 1) * 128],
                            rhs=pn[:, mc, :],
                            start=(mc == 0), stop=(mc == NC - 1))
                nc.scalar.activation(
                    at[:, hi].rearrange("p a b -> p (a b)"), opss[hi][:],
                    AF.Exp)
            # elu(x) = relu(x) + min(exp(x)-1, 0); relu on scalar (slack),
            # fix and combine on vector at 2x bf16 rate
            relu = hpool.tile([128, 2, NC, O], bf16, tag="relu")
            for hi in range(2):
                nc.scalar.activation(
                    relu[:, hi].rearrange("p a b -> p (a b)"), opss[hi][:],
                    AF.Relu)
            nc.vector.tensor_scalar(
                at[:].rearrange("p a b c -> p (a b c)"),
                at[:].rearrange("p a b c -> p (a b c)"),
                1.0, 0.0, op0=AL.subtract, op1=AL.min)
            for hi in range(2):
                h = 2 * hp + hi
                nc.vector.tensor_tensor(
                    hT[:, h * NC:(h + 1) * NC, :],
                    at[:, hi], relu[:, hi], op=AL.add)

        def emit_l2(g):
            s = G[g]
            npm_sb, mnegt, hT = s["npm"], s["mnegt"], s["hT"]
            wh2 = gpool.tile([128, NC, OUT], bf16, tag="wh2")
            e12f = gpool.tile([128, NC, 2], f32, tag="e12")
            for c in range(NC):
                for half in range(2):
                    hps = ps_a.tile([128, 258], f32, tag="aug")
                    for k in range(KC2):
                        nc.tensor.matmul(
                            hps[:], lhsT=hT[:, k, c * 128:(c + 1) * 128],
                            rhs=wo_sb[:, k, half, :],
                            start=(k == 0), stop=(k == KC2 - 1))
                    nc.scalar.activation(
                        wh2[:, c, half * 256:(half + 1) * 256], hps[:, 0:256],
                        AF.Copy, scale=npm_sb[:, c:c + 1])
                    nc.vector.tensor_scalar(
                        e12f[:, c, half:half + 1], hps[:, 256:257],
                        npm_sb[:, c:c + 1], None, op0=AL.mult)
            # e1o columns -> broadcast rows via DRAM round-trip (no PE needed)
            e1cb = gpool.tile([128, NC], bf16, tag="e1cb")
            nc.vector.tensor_copy(e1cb[:], e12f[:, :, 0])
            es = zscr_d[g, H]
            nc.sync.dma_start(
                bass.AP(tensor=es.tensor, offset=es.offset,
                        ap=[[1, 128], [128, NC]]), e1cb[:])
            e1b2 = gpool.tile([128, N], bf16, tag="e1b2")
            nc.sync.dma_start(
                e1b2[:], bass.AP(tensor=es.tensor, offset=es.offset,
                                 ap=[[0, 128], [1, N]]))
            x2 = hpool.tile([128, NC, N], bf16, tag="x2")
            for mc in range(NC):
                nc.vector.scalar_tensor_tensor(
                    x2[:, mc, :], mnegt[:, mc, :], e12f[:, mc, 1:2],
                    e1b2[:], op0=AL.add, op1=AL.add)
            zt2 = hpool.tile([128, NC, N], bf16, tag="zt2")
            nc.vector.scalar_tensor_tensor(
                zt2[:].rearrange("p a b -> p (a b)"),
                x2[:].rearrange("p a b -> p (a b)"), ALPHA,
                x2[:].rearrange("p a b -> p (a b)"),
                op0=AL.mult, op1=AL.max)
            p2 = hpool.tile([128, NC, N], bf16, tag="pu2")
            nc.scalar.activation(p2[:].rearrange("p a b -> p (a b)"),
                                 zt2[:].rearrange("p a b -> p (a b)"), AF.Exp)
            zc2 = ps_z.tile([128, NC], f32, tag="z")
            for c in range(NC):
                for mc in range(NC):
                    nc.tensor.matmul(
                        zc2[:, c:c + 1],
                        lhsT=p2[:, mc, c * 128:(c + 1) * 128],
                        rhs=ones_col[:], start=(mc == 0), stop=(mc == NC - 1))
            zic2 = hpool.tile([128, NC], bf16, tag="zic2")
            with nc.allow_low_precision("bf16 softmax 1/z"):
                nc.vector.reciprocal(zic2[:], zc2[:])
            # fold the output-row non-pad mask into 1/z (both column layout)
            nc.vector.tensor_mul(zic2[:], zic2[:], npm_sb[:])
            zs2 = zscr_d[g, H + 1]
            nc.sync.dma_start(
                bass.AP(tensor=zs2.tensor, offset=zs2.offset,
                        ap=[[1, 128], [128, NC]]), zic2[:])
            zbc2s = hpool.tile([128, N], bf16, tag="zbc2s")
            nc.sync.dma_start(
                zbc2s[:], bass.AP(tensor=zs2.tensor, offset=zs2.offset,
                                  ap=[[0, 128], [1, N]]))
            pn2 = hpool.tile([128, NC, N], bf16, tag="pn2")
            nc.vector.tensor_tensor(pn2[:], p2[:], bc_nc(zbc2s[:]), op=AL.mult)
            out_sb = gpool.tile([128, NC, OUT], bf16, tag="osb")
            a2 = gpool.tile([128, NC, OUT], bf16, tag="a2")
            r2 = gpool.tile([128, NC, OUT], bf16, tag="r2")
            for c in range(NC):
                o2ps = ps_b.tile([128, OUT], f32, tag="big")
                for mc in range(NC):
                    nc.tensor.matmul(
                        o2ps[:], lhsT=pn2[:, mc, c * 128:(c + 1) * 128],
                        rhs=wh2[:, mc, :], start=(mc == 0), stop=(mc == NC - 1))
                nc.scalar.activation(a2[:, c, :], o2ps[:], AF.Exp)
                nc.scalar.activation(r2[:, c, :], o2ps[:], AF.Relu)
            nc.vector.tensor_scalar(
                a2[:].rearrange("p a b -> p (a b)"),
                a2[:].rearrange("p a b -> p (a b)"),
                1.0, 0.0, op0=AL.subtract, op1=AL.min)
            nc.vector.tensor_tensor(
                out_sb[:].rearrange("p a b -> p (a b)"),
                a2[:].rearrange("p a b -> p (a b)"),
                r2[:].rearrange("p a b -> p (a b)"), op=AL.add)
            nc.gpsimd.dma_start(out_d[g], out_sb[:])
            del G[g]

        # per-graph emission, interleaved in pairs; the previous pair's
        # layer-2 is drained INSIDE the next pair's head loop so the tensor
        # engine never sees a serial layer-2-only stretch.
        for gp in range(GPC // 2):
            g0, g1 = 2 * gp, 2 * gp + 1
            emit_setup(g0)
            emit_setup(g1)
            emit_aug(g0, 0)
            emit_aug(g1, 0)
            for hp in range(HP):
                for g in (g0, g1):
                    if hp + 1 < HP:
                        emit_aug(g, hp + 1)
                    emit_head_pair(g, hp)
            emit_l2(g0)
            emit_l2(g1)

    nc.compile()
    return nc


_NC_CACHE = {}


def build_kernel():
    if "nc" not in _NC_CACHE:
        _NC_CACHE["nc"] = _build_nc()
    return _NC_CACHE["nc"]


def _host_prep(fea, adj, non_pad_mask, embed, W_heads, a_heads, W_out, a_out):
    """Fold attention vectors into weights (f64) and pre-layout per-core inputs."""
    W64 = W_heads.astype(np.float64)
    w1 = np.einsum("hfo,ho->hf", W64, a_heads[:, :O].astype(np.float64))
    w2 = np.einsum("hfo,ho->hf", W64, a_heads[:, O:].astype(np.float64))
    emb64 = np.zeros((VC * 128, F))
    emb64[:VOCAB] = embed.astype(np.float64)
    # embW[h] = embed @ W[h] -> (256, O); heads packed in pairs on free axis
    embw = np.einsum("vf,hfo->hvo", emb64, W64)                  # (H, 256, O)
    embw = embw.reshape(HP, 2, VC, 128, O).transpose(3, 2, 0, 1, 4)
    embw = np.ascontiguousarray(embw.reshape(128, VC, HP, 2 * O)).astype(BF)
    # layer-1 attention logits are pure gathers of host tables
    t1 = emb64 @ w1.T                                            # (256, H)
    t2 = emb64 @ w2.T
    e1 = t1[fea]                                                 # (B, N, H)
    e2 = t2[fea]
    # e2 as per-partition columns (m on partitions), f32
    e2c = np.ascontiguousarray(
        e2.reshape(B, NC, 128, H).transpose(0, 2, 1, 3)).astype(np.float32)
    # e1 as rows replicated across partitions, bf16
    e1bc = np.ascontiguousarray(
        np.broadcast_to(e1.transpose(0, 2, 1)[:, None, :, :],
                        (B, 128, H, N))).astype(BF)

    Wo64 = W_out.astype(np.float64)
    w1o = Wo64 @ a_out[:OUT].astype(np.float64)
    w2o = Wo64 @ a_out[OUT:].astype(np.float64)
    zcol = np.zeros((H * O, 1))
    woaug = np.concatenate(
        [Wo64[:, 0:256], w1o[:, None], zcol,
         Wo64[:, 256:512], w2o[:, None], zcol], axis=1)       # (2048, 516)
    woaug = np.ascontiguousarray(
        woaug.reshape(KC2, 128, 2, 258).transpose(1, 0, 2, 3)).astype(BF)

    vidx = np.arange(VC * 128).reshape(VC, 128)
    oh = (fea[:, None, None, :] == vidx[None, :, :, None])       # (B, VC, 128, N)
    oh = np.ascontiguousarray(oh.transpose(0, 2, 1, 3)).astype(BF)

    # mask in M-MAJOR: mnegT[m, n] built from adj^T
    adjT = adj.transpose(0, 2, 1).reshape(B, NC, 128, N).transpose(0, 2, 1, 3)
    mnegt = np.ascontiguousarray(
        np.where(adjT > 0, 0.0, MASK_NEG).astype(BF))
    npm = np.ascontiguousarray(
        non_pad_mask.reshape(B, NC, 128).transpose(0, 2, 1)).astype(np.float32)

    return oh, mnegt, e2c, e1bc, npm, embw, woaug


def kernel(fea, adj, non_pad_mask, embed, W_heads, a_heads, W_out, a_out,
           _mm_dt=None, _trace=False):
    oh, mnegt, e2c, e1bc, npm, embw, woaug = _host_prep(
        fea, adj, non_pad_mask, embed, W_heads, a_heads, W_out, a_out)

    nc = build_kernel()
    identity = np.eye(128, dtype=np.float32)
    in_maps = []
    for i in range(NCORES):
        sl = slice(i * GPC, (i + 1) * GPC)
        in_maps.append({
            "oh": oh[sl], "mnegt": mnegt[sl], "e2c": e2c[sl], "e1bc": e1bc[sl],
            "npm": npm[sl], "embw2": embw,
            "woaug": woaug, "identity": identity,
        })
    res = run_bass_kernel_spmd(nc, in_maps, core_ids=list(range(NCORES)),
                               trace=_trace)
    outs = []
    for i in range(NCORES):
        o = np.asarray(res.results[i]["out"]).astype(np.float32)
        outs.append(o.transpose(0, 2, 1, 3).reshape(GPC, N, OUT))
    full = np.concatenate(outs, axis=0)
    if _trace:
        kernel.last_results = res
    return full
